# revision 1
# baseline (speedup 1.0000x reference)
"""GCN message-passing kernel for Trainium2 (8 NeuronCores, SPMD).

out = (D^-1/2 (A+I) D^-1/2 X) W^T + b   for a random graph with
N=100000 nodes, E=1600000 edges, 128 channels.

Strategy (per core; destinations sharded 12500 nodes/core):
- Every edge becomes a "token" with coefficient gamma = dinv[src]*dinv[dst];
  aggregation for a window of 128 destinations is
  aggT[ch, dst] = sum_tokens x[src]^T * onehot,
  onehot[e, d] = gamma[e] * (destrel[e] == d), computed as fp32 matmuls
  accumulated in PSUM (lhsT = gathered message tile, rhs = one-hot tile).
- Gathers use dma_gather (SWDGE, int16 indices) against 4 banked views of x
  (32768 rows each); calls round-robin over 4 SWDGE queues so descriptor
  generation runs on all 8 GpSimd cores in parallel.
- Self-loops skip the gather: each window's own x rows load with a plain
  sequential DMA and enter the same one-hot matmul path.
- Token order: [super of 16 windows][source bank][window][128-token tiles],
  padded with gamma=0 tokens so all 8 cores share one instruction stream.
- Finalize per window: outT = W^T @ aggT + b, written as outT[128, 12544]
  per core; host transposes/assembles.
"""

import sys

sys.path.insert(0, "/opt/trn_rl_repo")
import numpy as np

N = 100000
D = 128
CORES = 8
NPC = N // CORES  # 12500 dests per core
NW = (NPC + 127) // 128  # 98 windows per core
SUP = 6  # windows per super (PSUM accumulator banks: 6 + 2 for out matmul)
NSUP = (NW + SUP - 1) // SUP  # 7 supers
BANK = 32768
NBANKS = (N + BANK - 1) // BANK  # 4
CHUNK = 16  # gather-call size in 128-token tiles


def _build_bass(tiles, sup_windows):
    """Build the SPMD Bass program.

    tiles: int array [NSUP, NBANKS, NW] - tile count per group (global).
    sup_windows: list of per-super window lists.
    """
    import os

    import concourse.mybir as mybir
    import concourse.tile as tile
    from concourse import bacc

    lim_sup = int(os.environ.get("K_LIMIT_SUPERS", NSUP))
    T_total = int(tiles.sum())
    NTOK = 128 * T_total

    nc = bacc.Bacc(None, target_bir_lowering=False, num_swdge_queues=4)
    xt = nc.dram_tensor("xt", [N, D], mybir.dt.float32, kind="ExternalInput")
    idxs = nc.dram_tensor("idxs", [128, NTOK // 16], mybir.dt.int16, kind="ExternalInput")
    destrel = nc.dram_tensor("destrel", [128, T_total], mybir.dt.float32, kind="ExternalInput")
    gamma = nc.dram_tensor("gamma", [128, T_total], mybir.dt.float32, kind="ExternalInput")
    gself = nc.dram_tensor("gself", [128, NW], mybir.dt.float32, kind="ExternalInput")
    wt = nc.dram_tensor("wt", [D, D], mybir.dt.float32, kind="ExternalInput")
    bvec = nc.dram_tensor("bvec", [D, 1], mybir.dt.float32, kind="ExternalInput")
    outT = nc.dram_tensor("outT", [D, NW * 128], mybir.dt.float32, kind="ExternalOutput")

    xviews = [xt[b * BANK : min((b + 1) * BANK, N), :] for b in range(NBANKS)]

    # per-core compact x rows for self-loop loads (sequential DMA)
    xself_in = nc.dram_tensor("xself", [NW * 128, D], mybir.dt.float32, kind="ExternalInput")

    gq = [0]
    OHK = 16  # one-hot batch, in tiles
    with tile.TileContext(nc) as tc:
        with (
            tc.tile_pool(name="const", bufs=1) as cpool,
            tc.tile_pool(name="meta", bufs=1) as mpool,
            tc.tile_pool(name="gp", bufs=8) as gpool,
            tc.tile_pool(name="ohp", bufs=4) as ohpool,
            tc.tile_pool(name="sohp", bufs=2) as sohpool,
            tc.tile_pool(name="selfp", bufs=3) as selfpool,
            tc.tile_pool(name="rhp", bufs=3) as rhpool,
            tc.tile_pool(name="outp", bufs=2) as outpool,
            tc.tile_pool(name="idxp", bufs=8) as idxpool,
            tc.tile_pool(name="ps", bufs=1, space="PSUM") as pspool,
            tc.tile_pool(name="pso", bufs=2, space="PSUM") as psopool,
        ):
            wt_t = cpool.tile([D, D], mybir.dt.float32)
            nc.sync.dma_start(out=wt_t[:], in_=wt[:])
            b_t = cpool.tile([D, 1], mybir.dt.float32)
            nc.sync.dma_start(out=b_t[:], in_=bvec[:])
            iota_t = cpool.tile([128, 128], mybir.dt.float32)
            nc.gpsimd.iota(
                iota_t[:], pattern=[[1, 128]], base=0, channel_multiplier=0,
                allow_small_or_imprecise_dtypes=True,
            )
            pidx_t = cpool.tile([128, 1], mybir.dt.float32)
            nc.gpsimd.iota(
                pidx_t[:], pattern=[[1, 1]], base=0, channel_multiplier=1,
                allow_small_or_imprecise_dtypes=True,
            )
            # constant self one-hot base: (iota == p)
            selfbase_t = cpool.tile([128, 128], mybir.dt.float32)
            nc.vector.tensor_tensor(
                out=selfbase_t[:],
                in0=iota_t[:],
                in1=pidx_t[:, 0:1].to_broadcast([128, 128]),
                op=mybir.AluOpType.is_equal,
            )
            destrel_t = mpool.tile([128, T_total], mybir.dt.float32)
            nc.sync.dma_start(out=destrel_t[:], in_=destrel[:])
            gamma_t = mpool.tile([128, T_total], mybir.dt.float32)
            nc.sync.dma_start(out=gamma_t[:], in_=gamma[:])
            gself_t = mpool.tile([128, NW], mybir.dt.float32)
            nc.sync.dma_start(out=gself_t[:], in_=gself[:])

            oh_batches = {}

            def oh_for(gt):
                bnum = gt // OHK
                if bnum not in oh_batches:
                    t0 = bnum * OHK
                    k = min(OHK, T_total - t0)
                    ohb = ohpool.tile([128, OHK, 128], mybir.dt.float32, tag="ohb")
                    nc.vector.tensor_tensor(
                        out=ohb[:, :k, :],
                        in0=iota_t[:, None, :].to_broadcast([128, k, 128]),
                        in1=destrel_t[:, t0 : t0 + k, None].to_broadcast([128, k, 128]),
                        op=mybir.AluOpType.is_equal,
                    )
                    nc.vector.tensor_tensor(
                        out=ohb[:, :k, :],
                        in0=ohb[:, :k, :],
                        in1=gamma_t[:, t0 : t0 + k, None].to_broadcast([128, k, 128]),
                        op=mybir.AluOpType.mult,
                    )
                    oh_batches[bnum] = ohb
                    for old in list(oh_batches):
                        if old < bnum - 2:
                            del oh_batches[old]
                return oh_batches[bnum][:, gt % OHK, :]

            tile_cursor = 0  # global tile index in token order
            for S in range(NSUP):
                if S >= lim_sup:
                    break
                wins = sup_windows[S]
                nwin = len(wins)
                # batched self one-hots for this super
                soh = sohpool.tile([128, SUP, 128], mybir.dt.float32, tag="soh")
                nc.vector.tensor_tensor(
                    out=soh[:, :nwin, :],
                    in0=selfbase_t[:, None, :].to_broadcast([128, nwin, 128]),
                    in1=gself_t[:, wins[0] : wins[0] + nwin, None].to_broadcast(
                        [128, nwin, 128]
                    ),
                    op=mybir.AluOpType.mult,
                )
                psbank = {}
                mm_done = {w: 0 for w in wins}
                mm_total = {
                    w: 1 + int(sum(tiles[S, bb, w] for bb in range(NBANKS)))
                    for w in wins
                }
                for b in range(NBANKS):
                    region_tiles = int(sum(tiles[S, b, w] for w in wins))
                    chunk_tiles = []  # (start_tile_global, len, sbuf_tile)
                    c0 = 0
                    while c0 < region_tiles:
                        clen = min(CHUNK, region_tiles - c0)
                        gtile = gpool.tile([128, CHUNK, D], mybir.dt.float32, tag="g")
                        itile = idxpool.tile([128, CHUNK * 8], mybir.dt.int16, tag="ix")
                        gstart = tile_cursor + c0
                        nc.sync.dma_start(
                            out=itile[:, : clen * 8],
                            in_=idxs[:, gstart * 8 : (gstart + clen) * 8],
                        )
                        nc.gpsimd.dma_gather(
                            gtile[:, :clen, :],
                            xviews[b],
                            itile[:, : clen * 8],
                            128 * clen,
                            128 * clen,
                            D,
                            elem_step=D,
                            single_packet=False,
                            queue_num=gq[0] % 4,
                        )
                        gq[0] += 1
                        chunk_tiles.append((gstart, clen, gtile))
                        c0 += clen

                    def chunk_for(gt):
                        for cs, cl, ct in chunk_tiles:
                            if cs <= gt < cs + cl:
                                return ct, gt - cs
                        raise AssertionError

                    t_local = tile_cursor
                    for wi, w in enumerate(wins):
                        nt = int(tiles[S, b, w])
                        if b == 0:
                            # self-loop tile first: sequential x rows
                            ps = pspool.tile(
                                [128, 128], mybir.dt.float32, tag=f"psw{w % SUP}",
                                name=f"psw{S}_{w % SUP}",
                            )
                            psbank[w] = ps
                            xs = selfpool.tile([128, D], mybir.dt.float32, tag="xs")
                            nc.sync.dma_start(
                                out=xs[:], in_=xself_in[w * 128 : (w + 1) * 128, :]
                            )
                            nc.tensor.matmul(
                                out=ps[:],
                                lhsT=xs[:],
                                rhs=soh[:, wi, :],
                                start=True,
                                stop=(mm_total[w] == 1),
                                skip_group_check=True,
                            )
                            mm_done[w] = 1
                        for t in range(nt):
                            gt = t_local + t
                            ctile, ccol = chunk_for(gt)
                            nc.tensor.matmul(
                                out=psbank[w][:],
                                lhsT=ctile[:, ccol, :],
                                rhs=oh_for(gt),
                                start=False,
                                stop=(mm_done[w] == mm_total[w] - 1),
                                skip_group_check=True,
                            )
                            mm_done[w] += 1
                        t_local += nt
                    tile_cursor += region_tiles

                # finalize super: outT_w = W^T @ aggT_w + b
                ostage = outpool.tile([128, SUP * 128], mybir.dt.float32, tag="ostage")
                for wi, w in enumerate(wins):
                    rh = rhpool.tile([128, 128], mybir.dt.float32, tag="rh")
                    nc.vector.tensor_copy(out=rh[:], in_=psbank[w][:])
                    mm = psopool.tile([D, 128], mybir.dt.float32, tag="po")
                    nc.tensor.matmul(
                        out=mm[:], lhsT=wt_t[:], rhs=rh[:], start=True, stop=True
                    )
                    nc.scalar.activation(
                        out=ostage[:, wi * 128 : (wi + 1) * 128],
                        in_=mm[:],
                        func=mybir.ActivationFunctionType.Identity,
                        bias=b_t[:, 0:1],
                        scale=1.0,
                    )
                nc.sync.dma_start(
                    out=outT[:, wins[0] * 128 : (wins[-1] + 1) * 128],
                    in_=ostage[:, : len(wins) * 128],
                )

    nc.finalize()
    return nc


def _preprocess(x, edge_index, W, b):
    """Host-side sharding: build per-core token tables + global schedule."""
    row = np.asarray(edge_index[0], dtype=np.int64)
    col = np.asarray(edge_index[1], dtype=np.int64)
    deg = (np.bincount(col, minlength=N) + 1).astype(np.float32)
    dinv = deg**-0.5  # float32, deg >= 1 always

    gam = (dinv[col] * dinv[row]).astype(np.float32)

    core = row // NPC
    lrow = row - core * NPC
    w = lrow // 128
    drel = (lrow % 128).astype(np.float32)
    S = w // SUP
    beta = col // BANK
    crel = (col - beta * BANK).astype(np.int16)

    # sort tokens by (core, S, beta, w)
    order = np.lexsort((w, beta, S, core))
    core_s = core[order]
    S_s = S[order]
    beta_s = beta[order]
    w_s = w[order]
    drel_s = drel[order]
    crel_s = crel[order]
    gam_s = gam[order]

    gid = ((core_s * NSUP + S_s) * NBANKS + beta_s) * NW + w_s
    NG = CORES * NSUP * NBANKS * NW
    counts = np.bincount(gid, minlength=NG).reshape(CORES, NSUP, NBANKS, NW)
    tiles = (counts + 127) // 128
    tiles_g = tiles.max(axis=0)  # [NSUP, NBANKS, NW]
    for s in range(NSUP):
        mask = np.zeros(NW, dtype=bool)
        mask[s * SUP : min((s + 1) * SUP, NW)] = True
        tiles_g[s, :, ~mask] = 0

    sup_windows = [list(range(s * SUP, min((s + 1) * SUP, NW))) for s in range(NSUP)]

    base = np.zeros((NSUP, NBANKS, NW), dtype=np.int64)
    cur = 0
    for s in range(NSUP):
        for bb in range(NBANKS):
            for ww in sup_windows[s]:
                base[s, bb, ww] = cur
                cur += 128 * int(tiles_g[s, bb, ww])
    T_total = cur // 128
    NTOK = cur

    Wt = np.ascontiguousarray(np.asarray(W, dtype=np.float32).T)
    bv = np.asarray(b, dtype=np.float32)[:, None].copy()
    xf = np.ascontiguousarray(np.asarray(x, dtype=np.float32))

    gid_full = core_s * (NSUP * NBANKS * NW) + (S_s * NBANKS + beta_s) * NW + w_s
    uniq, first_idx, cnt = np.unique(gid_full, return_index=True, return_counts=True)
    rank = np.arange(len(gid_full)) - np.repeat(first_idx, cnt)
    pos = base[S_s, beta_s, w_s] + rank

    core_bounds = np.searchsorted(core_s, np.arange(CORES + 1))
    in_maps = []
    for k in range(CORES):
        lo, hi = core_bounds[k], core_bounds[k + 1]
        idx16 = np.zeros(NTOK, dtype=np.int16)
        dr = np.zeros(NTOK, dtype=np.float32)
        gm = np.zeros(NTOK, dtype=np.float32)
        p = pos[lo:hi]
        idx16[p] = crel_s[lo:hi]
        dr[p] = drel_s[lo:hi]
        gm[p] = gam_s[lo:hi]
        idx_tile = np.tile(idx16.reshape(-1, 16).T, (8, 1))  # [128, NTOK//16]
        dr_t = np.ascontiguousarray(dr.reshape(T_total, 128).T)
        gm_t = np.ascontiguousarray(gm.reshape(T_total, 128).T)

        # self tables: gamma_self[p, w] = dinv[core row]^2 (0 beyond NPC)
        gs = np.zeros(NW * 128, dtype=np.float32)
        rows = np.arange(NPC) + k * NPC
        gs[:NPC] = dinv[rows] * dinv[rows]
        gs_t = np.ascontiguousarray(gs.reshape(NW, 128).T)
        # compact per-core x rows for self loads, padded to NW*128
        xs = np.zeros((NW * 128, D), dtype=np.float32)
        xs[:NPC] = xf[k * NPC : (k + 1) * NPC]
        in_maps.append(
            {
                "xt": xf,
                "idxs": idx_tile,
                "destrel": dr_t,
                "gamma": gm_t,
                "gself": gs_t,
                "xself": xs,
                "wt": Wt,
                "bvec": bv,
            }
        )

    return tiles_g, sup_windows, in_maps


_CACHE = {}


def kernel(x, edge_index, W, b, _want_trace=False):
    from concourse.bass_utils import run_bass_kernel_spmd

    tiles_g, sup_windows, in_maps = _preprocess(x, edge_index, W, b)
    key = tiles_g.tobytes()
    if key not in _CACHE:
        _CACHE[key] = _build_bass(tiles_g, sup_windows)
    nc = _CACHE[key]

    kwargs = {}
    if _want_trace:
        kwargs = dict(trace=True, trace_cores=list(range(CORES)))
    res = run_bass_kernel_spmd(nc, in_maps, core_ids=list(range(CORES)), **kwargs)

    out = np.empty((N, D), dtype=np.float32)
    for k in range(CORES):
        out[k * NPC : (k + 1) * NPC] = res.results[k]["outT"][:, :NPC].T
    if _want_trace:
        return out, res
    return out



# revision 2
# speedup vs baseline: 1.5877x; 1.5877x over previous
"""GCN message-passing kernel for Trainium2 (8 NeuronCores, SPMD).

out = (D^-1/2 (A+I) D^-1/2 X) W^T + b   for a random graph with
N=100000 nodes, E=1600000 edges, 128 channels.

Strategy (per core; destinations sharded 12500 nodes/core):
- Host pre-scales rows: xs[n] = dinv[n] * x[n] (bf16). Each edge token
  gathers xs[src]; aggregation for a window of 128 destinations is
  aggT[ch, dst] = sum_tok xs[src]^T * onehot, with a BINARY one-hot
  (destrel == iota), accumulated fp32 in PSUM via bf16 matmuls.
  The remaining dinv[dst] factor is applied during the PSUM->SBUF copy
  against a replicated per-window dinv table; padding tokens carry
  destrel = -1 so their one-hot row is all zero.
- Gathers use dma_gather (SWDGE, int16 indices) against 4 banked views
  of xs (32768 rows each), 2048 tokens per call, round-robin over the
  4 SWDGE queues (per-queue packet service ~8ns is the kernel's floor).
- Self-loops skip the gather: each window's own xs rows load with a
  plain sequential DMA and a constant self one-hot.
- Token order: [super of 6 windows][source bank][window][128-token
  tiles], padded so all 8 cores share one instruction stream.
- Finalize per window: rh = aggT * dinv_w (bf16), outT = W^T @ rh + b,
  written as outT[128, 12544] per core; host transposes/assembles.
"""

import sys

sys.path.insert(0, "/opt/trn_rl_repo")
import numpy as np
import ml_dtypes

BF16 = ml_dtypes.bfloat16

N = 100000
D = 128
CORES = 8
NPC = N // CORES  # 12500 dests per core
NW = (NPC + 127) // 128  # 98 windows per core
SUP = 6  # windows per super (PSUM accumulator banks: 6 + 2 for out matmul)
NSUP = (NW + SUP - 1) // SUP  # 17 supers of 6 windows
BANK = 32768
NBANKS = (N + BANK - 1) // BANK  # 4
CHUNK = 16  # gather-call size in 128-token tiles


def _build_bass(tiles, sup_windows):
    """Build the SPMD Bass program.

    tiles: int array [NSUP, NBANKS, NW] - tile count per group (global).
    sup_windows: list of per-super window lists.
    """
    import os

    import concourse.mybir as mybir
    import concourse.tile as tile
    from concourse import bacc

    lim_sup = int(os.environ.get("K_LIMIT_SUPERS", NSUP))
    T_total = int(tiles.sum())
    NTOK = 128 * T_total

    nc = bacc.Bacc(None, target_bir_lowering=False, num_swdge_queues=4)
    xt = nc.dram_tensor("xt", [N, D], mybir.dt.bfloat16, kind="ExternalInput")
    idxs = nc.dram_tensor("idxs", [128, NTOK // 16], mybir.dt.int16, kind="ExternalInput")
    destrel = nc.dram_tensor("destrel", [128, T_total], mybir.dt.bfloat16, kind="ExternalInput")
    dinvw = nc.dram_tensor("dinvw", [128, NW * 128], mybir.dt.bfloat16, kind="ExternalInput")
    wt = nc.dram_tensor("wt", [D, D], mybir.dt.bfloat16, kind="ExternalInput")
    bvec = nc.dram_tensor("bvec", [D, 1], mybir.dt.float32, kind="ExternalInput")
    outT = nc.dram_tensor("outT", [D, NW * 128], mybir.dt.float32, kind="ExternalOutput")

    xviews = [xt[b * BANK : min((b + 1) * BANK, N), :] for b in range(NBANKS)]

    # per-core compact pre-scaled xs rows for self-loop loads (sequential DMA)
    xself_in = nc.dram_tensor("xself", [NW * 128, D], mybir.dt.bfloat16, kind="ExternalInput")

    gq = [0]
    OHK = 16  # one-hot batch, in tiles
    with tile.TileContext(nc) as tc:
        with (
            tc.tile_pool(name="const", bufs=1) as cpool,
            tc.tile_pool(name="meta", bufs=1) as mpool,
            tc.tile_pool(name="gp", bufs=12) as gpool,
            tc.tile_pool(name="ohp", bufs=4) as ohpool,
            tc.tile_pool(name="selfp", bufs=3) as selfpool,
            tc.tile_pool(name="rhp", bufs=3) as rhpool,
            tc.tile_pool(name="outp", bufs=2) as outpool,
            tc.tile_pool(name="ps", bufs=1, space="PSUM") as pspool,
            tc.tile_pool(name="pso", bufs=2, space="PSUM") as psopool,
        ):
            wt_t = cpool.tile([D, D], mybir.dt.bfloat16)
            nc.sync.dma_start(out=wt_t[:], in_=wt[:])
            b_t = cpool.tile([D, 1], mybir.dt.float32)
            nc.sync.dma_start(out=b_t[:], in_=bvec[:])
            iota_f = cpool.tile([128, 128], mybir.dt.float32)
            nc.gpsimd.iota(
                iota_f[:], pattern=[[1, 128]], base=0, channel_multiplier=0,
                allow_small_or_imprecise_dtypes=True,
            )
            pidx_f = cpool.tile([128, 1], mybir.dt.float32)
            nc.gpsimd.iota(
                pidx_f[:], pattern=[[1, 1]], base=0, channel_multiplier=1,
                allow_small_or_imprecise_dtypes=True,
            )
            iota_t = cpool.tile([128, 128], mybir.dt.bfloat16)
            nc.vector.tensor_copy(out=iota_t[:], in_=iota_f[:])
            # constant self one-hot: (iota == p), bf16
            selfbase_t = cpool.tile([128, 128], mybir.dt.bfloat16)
            nc.vector.tensor_tensor(
                out=selfbase_t[:],
                in0=iota_f[:],
                in1=pidx_f[:, 0:1].to_broadcast([128, 128]),
                op=mybir.AluOpType.is_equal,
            )
            destrel_t = mpool.tile([128, T_total], mybir.dt.bfloat16)
            nc.sync.dma_start(out=destrel_t[:], in_=destrel[:])
            dinvw_t = mpool.tile([128, NW * 128], mybir.dt.bfloat16)
            nc.sync.dma_start(out=dinvw_t[:], in_=dinvw[:])
            # whole token-index table stays resident
            itile = mpool.tile([128, NTOK // 16], mybir.dt.int16)
            nc.sync.dma_start(out=itile[:], in_=idxs[:])

            oh_batches = {}

            def oh_for(gt):
                bnum = gt // OHK
                if bnum not in oh_batches:
                    t0 = bnum * OHK
                    k = min(OHK, T_total - t0)
                    ohb = ohpool.tile([128, OHK, 128], mybir.dt.bfloat16, tag="ohb")
                    nc.vector.tensor_tensor(
                        out=ohb[:, :k, :],
                        in0=iota_t[:, None, :].to_broadcast([128, k, 128]),
                        in1=destrel_t[:, t0 : t0 + k, None].to_broadcast([128, k, 128]),
                        op=mybir.AluOpType.is_equal,
                    )
                    oh_batches[bnum] = ohb
                    for old in list(oh_batches):
                        if old < bnum - 2:
                            del oh_batches[old]
                return oh_batches[bnum][:, gt % OHK, :]

            tile_cursor = 0  # global tile index in token order
            for S in range(NSUP):
                if S >= lim_sup:
                    break
                wins = sup_windows[S]
                psbank = {}
                mm_done = {w: 0 for w in wins}
                mm_total = {
                    w: 1 + int(sum(tiles[S, bb, w] for bb in range(NBANKS)))
                    for w in wins
                }
                for b in range(NBANKS):
                    region_tiles = int(sum(tiles[S, b, w] for w in wins))
                    chunk_tiles = []  # (start_tile_global, len, sbuf_tile)
                    c0 = 0
                    while c0 < region_tiles:
                        clen = min(CHUNK, region_tiles - c0)
                        gtile = gpool.tile([128, CHUNK, D], mybir.dt.bfloat16, tag="g")
                        gstart = tile_cursor + c0
                        nc.gpsimd.dma_gather(
                            gtile[:, :clen, :],
                            xviews[b],
                            itile[:, gstart * 8 : (gstart + clen) * 8],
                            128 * clen,
                            128 * clen,
                            D,
                            elem_step=D,
                            single_packet=False,
                            queue_num=gq[0] % 4,
                        )
                        gq[0] += 1
                        chunk_tiles.append((gstart, clen, gtile))
                        c0 += clen

                    def chunk_for(gt):
                        for cs, cl, ct in chunk_tiles:
                            if cs <= gt < cs + cl:
                                return ct, gt - cs
                        raise AssertionError

                    t_local = tile_cursor
                    for wi, w in enumerate(wins):
                        nt = int(tiles[S, b, w])
                        if b == 0:
                            # self-loop tile first: sequential xs rows
                            ps = pspool.tile(
                                [128, 128], mybir.dt.float32, tag=f"psw{w % SUP}",
                                name=f"psw{S}_{w % SUP}",
                            )
                            psbank[w] = ps
                            xs = selfpool.tile([128, D], mybir.dt.bfloat16, tag="xs")
                            nc.sync.dma_start(
                                out=xs[:], in_=xself_in[w * 128 : (w + 1) * 128, :]
                            )
                            nc.tensor.matmul(
                                out=ps[:],
                                lhsT=xs[:],
                                rhs=selfbase_t[:],
                                start=True,
                                stop=(mm_total[w] == 1),
                                skip_group_check=True,
                            )
                            mm_done[w] = 1
                        for t in range(nt):
                            gt = t_local + t
                            ctile, ccol = chunk_for(gt)
                            nc.tensor.matmul(
                                out=psbank[w][:],
                                lhsT=ctile[:, ccol, :],
                                rhs=oh_for(gt),
                                start=False,
                                stop=(mm_done[w] == mm_total[w] - 1),
                                skip_group_check=True,
                            )
                            mm_done[w] += 1
                        t_local += nt
                    tile_cursor += region_tiles

                # finalize super: rh = aggT * dinv_w ; outT_w = W^T @ rh + b
                ostage = outpool.tile([128, SUP * 128], mybir.dt.float32, tag="ostage")
                for wi, w in enumerate(wins):
                    rh = rhpool.tile([128, 128], mybir.dt.bfloat16, tag="rh")
                    nc.vector.tensor_tensor(
                        out=rh[:],
                        in0=psbank[w][:],
                        in1=dinvw_t[:, w * 128 : (w + 1) * 128],
                        op=mybir.AluOpType.mult,
                    )
                    mm = psopool.tile([D, 128], mybir.dt.float32, tag="po")
                    nc.tensor.matmul(
                        out=mm[:], lhsT=wt_t[:], rhs=rh[:], start=True, stop=True
                    )
                    nc.scalar.activation(
                        out=ostage[:, wi * 128 : (wi + 1) * 128],
                        in_=mm[:],
                        func=mybir.ActivationFunctionType.Identity,
                        bias=b_t[:, 0:1],
                        scale=1.0,
                    )
                nc.sync.dma_start(
                    out=outT[:, wins[0] * 128 : (wins[-1] + 1) * 128],
                    in_=ostage[:, : len(wins) * 128],
                )

    nc.finalize()
    return nc


def _preprocess(x, edge_index, W, b):
    """Host-side sharding: build per-core token tables + global schedule."""
    row = np.asarray(edge_index[0], dtype=np.int64)
    col = np.asarray(edge_index[1], dtype=np.int64)
    deg = (np.bincount(col, minlength=N) + 1).astype(np.float32)
    dinv = deg**-0.5  # float32, deg >= 1 always

    core = row // NPC
    lrow = row - core * NPC
    w = lrow // 128
    drel = (lrow % 128).astype(np.float32)
    S = w // SUP
    beta = col // BANK
    crel = (col - beta * BANK).astype(np.int16)

    # sort tokens by (core, S, beta, w)
    order = np.lexsort((w, beta, S, core))
    core_s = core[order]
    S_s = S[order]
    beta_s = beta[order]
    w_s = w[order]
    drel_s = drel[order]
    crel_s = crel[order]

    gid = ((core_s * NSUP + S_s) * NBANKS + beta_s) * NW + w_s
    NG = CORES * NSUP * NBANKS * NW
    counts = np.bincount(gid, minlength=NG).reshape(CORES, NSUP, NBANKS, NW)
    tiles = (counts + 127) // 128
    tiles_g = tiles.max(axis=0)  # [NSUP, NBANKS, NW]
    for s in range(NSUP):
        mask = np.zeros(NW, dtype=bool)
        mask[s * SUP : min((s + 1) * SUP, NW)] = True
        tiles_g[s, :, ~mask] = 0

    sup_windows = [list(range(s * SUP, min((s + 1) * SUP, NW))) for s in range(NSUP)]

    base = np.zeros((NSUP, NBANKS, NW), dtype=np.int64)
    cur = 0
    for s in range(NSUP):
        for bb in range(NBANKS):
            for ww in sup_windows[s]:
                base[s, bb, ww] = cur
                cur += 128 * int(tiles_g[s, bb, ww])
    T_total = cur // 128
    NTOK = cur

    # pre-scaled features: xs[n] = dinv[n] * x[n], bf16
    xf = np.asarray(x, dtype=np.float32) * dinv[:, None]
    xb = xf.astype(BF16)

    Wt = np.ascontiguousarray(np.asarray(W, dtype=np.float32).T.astype(BF16))
    bv = np.asarray(b, dtype=np.float32)[:, None].copy()

    gid_full = core_s * (NSUP * NBANKS * NW) + (S_s * NBANKS + beta_s) * NW + w_s
    uniq, first_idx, cnt = np.unique(gid_full, return_index=True, return_counts=True)
    rank = np.arange(len(gid_full)) - np.repeat(first_idx, cnt)
    pos = base[S_s, beta_s, w_s] + rank

    core_bounds = np.searchsorted(core_s, np.arange(CORES + 1))
    in_maps = []
    for k in range(CORES):
        lo, hi = core_bounds[k], core_bounds[k + 1]
        idx16 = np.zeros(NTOK, dtype=np.int16)
        dr = np.full(NTOK, -1.0, dtype=np.float32)  # padding: one-hot row = 0
        p = pos[lo:hi]
        idx16[p] = crel_s[lo:hi]
        dr[p] = drel_s[lo:hi]
        idx_tile = np.tile(idx16.reshape(-1, 16).T, (8, 1))  # [128, NTOK//16]
        dr_t = np.ascontiguousarray(dr.reshape(T_total, 128).T.astype(BF16))

        # per-window dinv columns, replicated across partitions
        dv = np.zeros(NW * 128, dtype=np.float32)
        dv[:NPC] = dinv[k * NPC : (k + 1) * NPC]
        dv_rep = np.ascontiguousarray(
            np.broadcast_to(dv[None, :].astype(BF16), (128, NW * 128)))
        # compact per-core xs rows for self loads, padded to NW*128
        xs = np.zeros((NW * 128, D), dtype=BF16)
        xs[:NPC] = xb[k * NPC : (k + 1) * NPC]
        in_maps.append(
            {
                "xt": xb,
                "idxs": idx_tile,
                "destrel": dr_t,
                "dinvw": dv_rep,
                "xself": xs,
                "wt": Wt,
                "bvec": bv,
            }
        )

    return tiles_g, sup_windows, in_maps


_CACHE = {}


def kernel(x, edge_index, W, b, _want_trace=False):
    from concourse.bass_utils import run_bass_kernel_spmd

    tiles_g, sup_windows, in_maps = _preprocess(x, edge_index, W, b)
    key = tiles_g.tobytes()
    if key not in _CACHE:
        _CACHE[key] = _build_bass(tiles_g, sup_windows)
    nc = _CACHE[key]

    kwargs = {}
    if _want_trace:
        kwargs = dict(trace=True, trace_cores=list(range(CORES)))
    res = run_bass_kernel_spmd(nc, in_maps, core_ids=list(range(CORES)), **kwargs)

    out = np.empty((N, D), dtype=np.float32)
    for k in range(CORES):
        out[k * NPC : (k + 1) * NPC] = res.results[k]["outT"][:, :NPC].T
    if _want_trace:
        return out, res
    return out
